# revision 35
# baseline (speedup 1.0000x reference)
"""Trainium2 Bass kernel for nn_AutoregressiveBisectionInverter.

Inverts y = softplus(s)*x + 0.1*x^3 + tanh(W@x + b) (W strictly lower
triangular) per batch row.  Since W is strictly lower-triangular, the tanh
term at position i depends only on already-solved x_{<i}; each position is
a monotone-cubic scalar root solve.

Strategy (per NeuronCore, batch sharded 1024 -> 8 x 128 rows on the 128
SBUF partitions):
  - Normalize:  x = sqrt(abar)*v with abar = 10*softplus(s)  so the cubic
    becomes p(v) = v^3 + v + dt  (unit coefficients, p' >= 1, |root| <= VM).
  - Per autoregressive step i (serial DVE chain + ScalarE leg):
      ScalarE: tanh_i = Tanh(W'[i,i-1]*v_{i-1} + cb)  -- the last dot term
               rides tanh's scale, cb = (partial dot + b_i) comes from a
               Copy+accum_out reduce seeded with bias=b_i/D;
               nd = Yt[:,i] - kappa_i*tanh_i  (Identity activation)
      DVE:  cnt = #{k: u_k < nd} + seed   (ONE tensor_scalar is_lt+accum over
              a host-baked grid u_k = p-poly(v_k); exact fp32 count ~ 7
              bisection steps)
            two Newton polish rounds, each as: Horner scan (den), reciprocal,
            Horner scan (num), multiply -- tensor_tensor_scan with a
            stride-0 free-axis broadcast of v evaluates 3v^2+1 and 2v^3+nd
            in one instruction each; round 1 runs in count units with the
            grid pitch h1 folded into the scan initial values.
      The [128,32] partial-dot multiply for row i+1 runs on DVE during step
      i's tanh window (column i of v is still zero there).
  - Output x = sqrt(abar)*v (one elementwise mult), DMA out.

Raw bass Blocks are used (TileContext's tail drain trips a sync-wait limit
in this walrus build), with explicit drain() between every same-engine
producer->consumer pair (DVE/ACT pipelines do not interlock RAW hazards).
All input-dependent scalars are baked as instruction immediates at trace
time; broadcasts/grids are precomputed on the host and DMA'd in dependency
order so compute starts after the first small loads.
"""

import numpy as np

B, D = 1024, 32
NCORES = 8
ROWS = B // NCORES  # 128 rows per core == SBUF partitions
N1 = 128            # bisection-grid points in the fused count op


def _softplus64(x):
    x = x.astype(np.float64)
    return np.log1p(np.exp(-np.abs(x))) + np.maximum(x, 0)


def build(y, W, s, b):
    """Build the SPMD Bass program; returns (nc, in_maps)."""
    from contextlib import ExitStack
    import concourse.bass as bass
    from concourse import mybir

    f32 = mybir.dt.float32
    Alu = mybir.AluOpType
    Act = mybir.ActivationFunctionType

    y = np.ascontiguousarray(np.asarray(y), dtype=np.float32)
    W64 = np.asarray(W, dtype=np.float64)
    s64 = np.asarray(s, dtype=np.float64)
    b64 = np.asarray(b, dtype=np.float64)

    # ---- host precompute ----
    abar = 10.0 * _softplus64(s64)                 # v-linear coefficient
    sqrt_abar = np.sqrt(abar)
    kappa = (10.0 * abar ** -1.5).astype(np.float32)     # per-step immediates
    Yt = (10.0 * y.astype(np.float64) * abar[None, :] ** -1.5).astype(np.float32)
    Wp = np.ascontiguousarray((W64 * sqrt_abar[None, :]).astype(np.float32))
    SA = sqrt_abar.astype(np.float32)[None, :]            # [1, D]
    BT = b64.astype(np.float32)[None, :]                  # [1, D] tanh bias

    dmax = 10.0 * (1.0 + np.abs(y).max(axis=0)) * abar ** -1.5
    VM = float(np.max(np.minimum(np.cbrt(dmax), dmax)) * 1.02 + 1e-3)
    H1 = float(np.float32(2 * VM / (N1 - 1)))
    VM = float(np.float32(VM))
    vk = (-VM + np.arange(N1, dtype=np.float64) * H1)
    UG = ((vk * vk + 1.0) * vk).astype(np.float32)[None, :]   # [1, N1] p-poly
    SEED = float(np.float32(-VM / H1 - 0.5))  # v0 = (count + SEED) * H1

    # Per-step threshold table A[r,i,k] = (Yt[r,i] - u_k)/kappa_i lets the
    # count op compare directly against tanh output (A > t), so the nd
    # affine moves off the critical chain.
    AT = ((Yt.astype(np.float64)[:, :, None]
           - UG.astype(np.float64)[0][None, None, :])
          / (10.0 * abar ** -1.5)[None, :, None]).astype(np.float32)  # [B,D,N1]
    HW = 3 * D + N1 + N1  # header also carries A[:,0,:]
    WPB = np.ascontiguousarray(np.broadcast_to(Wp[None, :, :], (ROWS, D, D)))

    # ---- build the SPMD Bass program (input-dependent immediates baked) ----
    nc = bass.Bass()
    hd_d = nc.dram_tensor("hdr", [ROWS, HW], f32, kind="ExternalInput")
    ar_d = nc.dram_tensor("arest", [ROWS, D - 1, N1], f32, kind="ExternalInput")
    wp_d = nc.dram_tensor("wpb", [ROWS, D, D], f32, kind="ExternalInput")
    xo_d = nc.dram_tensor("xout", [ROWS, D], f32, kind="ExternalOutput")

    def frep(ap, k):
        # broadcast a [P,1] AP along the free axis via stride 0
        return bass.AP(tensor=ap.tensor, offset=ap.offset,
                       ap=[list(ap.ap[0]), [0, k]])

    with ExitStack() as ctx:
        v = ctx.enter_context(nc.sbuf_tensor([ROWS, D], f32))       # v-space solution
        wp = ctx.enter_context(nc.sbuf_tensor([ROWS, D, D], f32))   # W' bcast
        hdr = ctx.enter_context(nc.sbuf_tensor([ROWS, HW], f32))
        ytt = hdr[:, 0:D]
        btt = hdr[:, D:2 * D]
        sat = hdr[:, 2 * D:3 * D]
        ugt = hdr[:, 3 * D:3 * D + N1]
        a0t = hdr[:, 3 * D + N1:3 * D + 2 * N1]
        art = ctx.enter_context(nc.sbuf_tensor([ROWS, D - 1, N1], f32))
        xo = ctx.enter_context(nc.sbuf_tensor([ROWS, D], f32))
        gs = ctx.enter_context(nc.sbuf_tensor([ROWS, N1], f32))     # count scratch
        prod = ctx.enter_context(nc.sbuf_tensor([ROWS, D], f32))
        junk = ctx.enter_context(nc.sbuf_tensor([ROWS, D], f32))
        c = ctx.enter_context(nc.sbuf_tensor([ROWS, 1], f32))
        t = ctx.enter_context(nc.sbuf_tensor([ROWS, 1], f32))
        cb = ctx.enter_context(nc.sbuf_tensor([ROWS, 1], f32))      # cpart + b_i
        cnt = ctx.enter_context(nc.sbuf_tensor([ROWS, 1], f32))
        ndt = ctx.enter_context(nc.sbuf_tensor([ROWS, 3], f32))     # [0,0,nd]
        dden = ctx.enter_context(nc.sbuf_tensor([ROWS, 2], f32))    # [0,1]
        scd = ctx.enter_context(nc.sbuf_tensor([ROWS, 2], f32))     # den scan out
        scn = ctx.enter_context(nc.sbuf_tensor([ROWS, 3], f32))     # num scan out
        r = ctx.enter_context(nc.sbuf_tensor([ROWS, 1], f32))
        v1 = ctx.enter_context(nc.sbuf_tensor([ROWS, 1], f32))
        s_dma = ctx.enter_context(nc.semaphore("s_dma"))
        s_dve = ctx.enter_context(nc.semaphore("s_dve"))
        s_act = ctx.enter_context(nc.semaphore("s_act"))
        s_gp = ctx.enter_context(nc.semaphore("s_gp"))
        s_r = ctx.enter_context(nc.semaphore("s_r"))
        s_v = ctx.enter_context(nc.semaphore("s_v"))
        s_nd = ctx.enter_context(nc.semaphore("s_nd"))
        s_dmb = ctx.enter_context(nc.semaphore("s_dmb"))
        block = ctx.enter_context(nc.Block())

        @block.sync
        def _(sync):
            sync.dma_start(out=wp[:, :, :], in_=wp_d[:, :, :]).then_inc(s_dmb, 16)
            # final store: wait for the vector chain's last inc
            sync.wait_ge(s_dve, 2)
            sync.dma_start(out=xo_d[:, :], in_=xo[:, :]).then_inc(s_dmb, 16)
            sync.wait_ge(s_dmb, 32)

        @block.gpsimd
        def _(gpsimd):
            gpsimd.dma_start(out=hdr[:, :], in_=hd_d[:, :]).then_inc(s_dma, 16)
            gpsimd.dma_start(out=art[:, 0:3, :],
                             in_=ar_d[:, 0:3, :]).then_inc(s_dma, 16)
            gpsimd.dma_start(out=art[:, 3:15, :],
                             in_=ar_d[:, 3:15, :]).then_inc(s_dma, 16)
            gpsimd.dma_start(out=art[:, 15:31, :],
                             in_=ar_d[:, 15:31, :]).then_inc(s_dma, 16)

        # NOTE: DVE/ACT pipelines do not interlock same-engine RAW hazards in
        # raw bass -- a dependent back-to-back op reads stale SBUF.  Every
        # producer->consumer edge needs a drain() (pipeline flush) between.
        @block.vector
        def _(vector):
            nc.vector.memset(v[:, :], 0.0)
            nc.vector.memset(c[:, :], 0.0)
            nc.vector.memset(ndt[:, :], 0.0)
            nc.vector.memset(dden[:, 0:1], 0.0)
            nc.vector.memset(dden[:, 1:2], 1.0)
            nc.vector.drain().then_inc(s_dve, 1)  # c_0 = 0 / const tiles ready
            vector.wait_ge(s_dma, 16)  # header (ytt/btt/sat/ugt) landed
            for i in range(D):
                if 1 <= i <= D - 2:
                    # speculative partial-dot multiply for row i+1; runs under
                    # tanh_i (column i of v is still zero).  The free-axis sum
                    # happens on the otherwise-idle ScalarE.
                    if i == 1:
                        vector.wait_ge(s_dmb, 16)  # W' landed
                    if i >= 2:
                        vector.wait_ge(s_r, i - 1)  # ScalarE consumed prod row i
                    nc.vector.tensor_mul(prod[:, :], v[:, :], wp[:, i + 1, :])
                    nc.vector.drain().then_inc(s_gp, 1)
                if i in (1, 4, 16):
                    vector.wait_ge(s_dma, 32 + 16 * (0 if i == 1 else
                                                     1 if i == 4 else 2))
                vector.wait_ge(s_act, i + 1)  # tanh_i done
                # count = #{A_ik > t} + SEED  (== #{u_k < nd}, exact count)
                asrc = a0t if i == 0 else art[:, i - 1, :]
                nc.vector.tensor_scalar(
                    out=gs[:, :], in0=asrc, scalar1=t[:, :],
                    scalar2=SEED, op0=Alu.is_gt, op1=Alu.add,
                    accum_out=cnt[:, :])
                nc.vector.drain()
                # Newton round 1 in count units (v0 = cnt*H1); Horner scans:
                #   den = (3*H1^2*cnt)*cnt + 1 ; num = ((2*H1^3*cnt)*cnt)*cnt + nd
                nc.vector.tensor_tensor_scan(
                    out=scd[:, :], data0=frep(cnt[:, 0:1], 2), data1=dden[:, :],
                    initial=float(3 * H1 * H1), op0=Alu.mult, op1=Alu.add)
                nc.vector.drain()
                nc.vector.reciprocal(out=r[:, :], in_=scd[:, 1:2])
                vector.wait_ge(s_nd, i + 1)  # nd affine (off-chain) done
                nc.vector.tensor_tensor_scan(
                    out=scn[:, :], data0=frep(cnt[:, 0:1], 3), data1=ndt[:, :],
                    initial=float(2 * H1 ** 3), op0=Alu.mult, op1=Alu.add)
                nc.vector.drain()
                nc.vector.tensor_mul(v1[:, :], scn[:, 2:3], r[:, :])
                nc.vector.drain()
                # Newton round 2 -> write v[:, i]
                nc.vector.tensor_tensor_scan(
                    out=scd[:, :], data0=frep(v1[:, 0:1], 2), data1=dden[:, :],
                    initial=3.0, op0=Alu.mult, op1=Alu.add)
                nc.vector.drain()
                nc.vector.reciprocal(out=r[:, :], in_=scd[:, 1:2])
                nc.vector.tensor_tensor_scan(
                    out=scn[:, :], data0=frep(v1[:, 0:1], 3), data1=ndt[:, :],
                    initial=2.0, op0=Alu.mult, op1=Alu.add)
                nc.vector.drain()
                nc.vector.tensor_mul(v[:, i:i + 1], scn[:, 2:3], r[:, :])
                if i <= D - 2:
                    nc.vector.drain().then_inc(s_v, 1)
                else:
                    nc.vector.drain()
            nc.vector.tensor_mul(xo[:, :], v[:, :], sat[:, :])
            nc.vector.drain().then_inc(s_dve, 1)

        @block.scalar
        def _(scalar):
            scalar.wait_ge(s_dma, 16)  # header landed
            for i in range(D):
                if i >= 2:
                    # cb = (partial dot of row i) + b_i : Copy+accum with the
                    # per-element bias b_i/D so the sum carries the tanh bias.
                    scalar.wait_ge(s_gp, i - 1)
                    nc.scalar.activation(
                        out=junk[:, :], in_=prod[:, :], func=Act.Copy,
                        bias=float(b64[i] / D), scale=1.0,
                        accum_out=cb[:, :])
                    nc.scalar.drain().then_inc(s_r, 1)
                # tanh_i; the last dot term W'[i,i-1]*v_{i-1} rides the scale
                if i == 0:
                    scalar.wait_ge(s_dve, 1)
                    nc.scalar.activation(
                        out=t[:, :], in_=c[:, :], func=Act.Tanh,
                        bias=btt[:, 0:1], scale=1.0)
                elif i == 1:
                    scalar.wait_ge(s_v, 1)
                    nc.scalar.activation(
                        out=t[:, :], in_=v[:, 0:1], func=Act.Tanh,
                        bias=btt[:, 1:2], scale=float(Wp[1, 0]))
                else:
                    scalar.wait_ge(s_v, i)
                    nc.scalar.activation(
                        out=t[:, :], in_=v[:, i - 1:i], func=Act.Tanh,
                        bias=cb[:, :], scale=float(Wp[i, i - 1]))
                nc.scalar.drain().then_inc(s_act, 1)
                # nd = Yt[:,i] - kappa_i * tanh(...), written into ndt[:,2];
                # runs under the DVE count+first-scan window
                nc.scalar.activation(
                    out=ndt[:, 2:3], in_=t[:, :], func=Act.Identity,
                    bias=ytt[:, i:i + 1], scale=float(-kappa[i]))
                nc.scalar.drain().then_inc(s_nd, 1)

    in_maps = []
    for c0 in range(NCORES):
        sl = slice(c0 * ROWS, (c0 + 1) * ROWS)
        hdr_np = np.concatenate([
            Yt[sl],
            np.broadcast_to(BT, (ROWS, D)),
            np.broadcast_to(SA, (ROWS, D)),
            np.broadcast_to(UG, (ROWS, N1)),
            AT[sl, 0, :],
        ], axis=1)
        in_maps.append({"hdr": np.ascontiguousarray(hdr_np),
                        "arest": np.ascontiguousarray(AT[sl, 1:, :]),
                        "wpb": WPB})
    return nc, in_maps


def kernel(y, W, s, b):
    from concourse.bass_utils import run_bass_kernel_spmd

    nc, in_maps = build(y, W, s, b)
    res = run_bass_kernel_spmd(nc, in_maps, list(range(NCORES))).results
    X = np.concatenate([res[c]["xout"] for c in range(NCORES)], axis=0)
    return X.astype(np.float32)


if __name__ == "__main__":
    rng = np.random.default_rng(0)
    y = rng.standard_normal((B, D)).astype(np.float32)
    W = np.tril(rng.standard_normal((D, D)), -1).astype(np.float32) * 0.5
    s = rng.standard_normal(D).astype(np.float32)
    b = rng.standard_normal(D).astype(np.float32)
    X = kernel(y=y, W=W, s=s, b=b)
    print("out", X.shape, X.dtype, X[0, :4])


# revision 36
# speedup vs baseline: 1.0532x; 1.0532x over previous
"""Trainium2 Bass kernel for nn_AutoregressiveBisectionInverter.

Inverts y = softplus(s)*x + 0.1*x^3 + tanh(W@x + b) (W strictly lower
triangular) per batch row.  Since W is strictly lower-triangular, the tanh
term at position i depends only on already-solved x_{<i}; each position is
a monotone-cubic scalar root solve.

Strategy (per NeuronCore, batch sharded 1024 -> 8 x 128 rows on the 128
SBUF partitions):
  - Normalize:  x = sqrt(abar)*v with abar = 10*softplus(s)  so the cubic
    becomes p(v) = v^3 + v + dt  (unit coefficients, p' >= 1, |root| <= VM).
  - Per autoregressive step i (serial DVE chain + ScalarE leg):
      ScalarE: tanh_i = Tanh(W'[i,i-1]*v_{i-1} + cb)  -- the last dot term
               rides tanh's scale, cb = (partial dot + b_i) comes from a
               Copy+accum_out reduce seeded with bias=b_i/D;
               nd = Yt[:,i] - kappa_i*tanh_i  (Identity activation)
      DVE:  cnt = #{k: u_k < nd} + seed   (ONE tensor_scalar is_lt+accum over
              a host-baked grid u_k = p-poly(v_k); exact fp32 count ~ 7
              bisection steps)
            two Newton polish rounds, each as: Horner scan (den), reciprocal,
            Horner scan (num), multiply -- tensor_tensor_scan with a
            stride-0 free-axis broadcast of v evaluates 3v^2+1 and 2v^3+nd
            in one instruction each; round 1 runs in count units with the
            grid pitch h1 folded into the scan initial values.
      The [128,32] partial-dot multiply for row i+1 runs on DVE during step
      i's tanh window (column i of v is still zero there).
  - Output x = sqrt(abar)*v (one elementwise mult), DMA out.

Raw bass Blocks are used (TileContext's tail drain trips a sync-wait limit
in this walrus build), with explicit drain() between every same-engine
producer->consumer pair (DVE/ACT pipelines do not interlock RAW hazards).
All input-dependent scalars are baked as instruction immediates at trace
time; broadcasts/grids are precomputed on the host and DMA'd in dependency
order so compute starts after the first small loads.
"""

import numpy as np

B, D = 1024, 32
NCORES = 8
ROWS = B // NCORES  # 128 rows per core == SBUF partitions
N1 = 128            # bisection-grid points in the fused count op


def _softplus64(x):
    x = x.astype(np.float64)
    return np.log1p(np.exp(-np.abs(x))) + np.maximum(x, 0)


def build(y, W, s, b):
    """Build the SPMD Bass program; returns (nc, in_maps)."""
    from contextlib import ExitStack
    import concourse.bass as bass
    from concourse import mybir

    f32 = mybir.dt.float32
    Alu = mybir.AluOpType
    Act = mybir.ActivationFunctionType

    y = np.ascontiguousarray(np.asarray(y), dtype=np.float32)
    W64 = np.asarray(W, dtype=np.float64)
    s64 = np.asarray(s, dtype=np.float64)
    b64 = np.asarray(b, dtype=np.float64)

    # ---- host precompute ----
    abar = 10.0 * _softplus64(s64)                 # v-linear coefficient
    sqrt_abar = np.sqrt(abar)
    kappa = (10.0 * abar ** -1.5).astype(np.float32)     # per-step immediates
    Yt = (10.0 * y.astype(np.float64) * abar[None, :] ** -1.5).astype(np.float32)
    Wp = np.ascontiguousarray((W64 * sqrt_abar[None, :]).astype(np.float32))
    SA = sqrt_abar.astype(np.float32)[None, :]            # [1, D]
    BT = b64.astype(np.float32)[None, :]                  # [1, D] tanh bias

    dmax = 10.0 * (1.0 + np.abs(y).max(axis=0)) * abar ** -1.5
    VM = float(np.max(np.minimum(np.cbrt(dmax), dmax)) * 1.02 + 1e-3)
    H1 = float(np.float32(2 * VM / (N1 - 1)))
    VM = float(np.float32(VM))
    vk = (-VM + np.arange(N1, dtype=np.float64) * H1)
    UG = ((vk * vk + 1.0) * vk).astype(np.float32)[None, :]   # [1, N1] p-poly
    SEED = float(np.float32(-VM / H1 - 0.5))  # v0 = (count + SEED) * H1

    # One header array per core: [ ytt | btt | sat | ugt ] columns, plus a
    # pre-broadcast W' -- exactly two input DMAs (DMA cost here is dominated
    # by the 128 per-partition descriptors, not bytes).
    HW = 3 * D + N1
    WPB = np.ascontiguousarray(np.broadcast_to(Wp[None, :, :], (ROWS, D, D)))

    # ---- build the SPMD Bass program (input-dependent immediates baked) ----
    nc = bass.Bass()
    hd_d = nc.dram_tensor("hdr", [ROWS, HW], f32, kind="ExternalInput")
    wp_d = nc.dram_tensor("wpb", [ROWS, D, D], f32, kind="ExternalInput")
    xo_d = nc.dram_tensor("xout", [ROWS, D], f32, kind="ExternalOutput")

    def frep(ap, k):
        # broadcast a [P,1] AP along the free axis via stride 0
        return bass.AP(tensor=ap.tensor, offset=ap.offset,
                       ap=[list(ap.ap[0]), [0, k]])

    with ExitStack() as ctx:
        v = ctx.enter_context(nc.sbuf_tensor([ROWS, D], f32))       # v-space solution
        wp = ctx.enter_context(nc.sbuf_tensor([ROWS, D, D], f32))   # W' bcast
        hdr = ctx.enter_context(nc.sbuf_tensor([ROWS, HW], f32))
        ytt = hdr[:, 0:D]
        btt = hdr[:, D:2 * D]
        sat = hdr[:, 2 * D:3 * D]
        ugt = hdr[:, 3 * D:3 * D + N1]
        xo = ctx.enter_context(nc.sbuf_tensor([ROWS, D], f32))
        gs = ctx.enter_context(nc.sbuf_tensor([ROWS, N1], f32))     # count scratch
        prod = ctx.enter_context(nc.sbuf_tensor([ROWS, D], f32))
        junk = ctx.enter_context(nc.sbuf_tensor([ROWS, D], f32))
        c = ctx.enter_context(nc.sbuf_tensor([ROWS, 1], f32))
        t = ctx.enter_context(nc.sbuf_tensor([ROWS, 1], f32))
        cb = ctx.enter_context(nc.sbuf_tensor([ROWS, 1], f32))      # cpart + b_i
        cnt = ctx.enter_context(nc.sbuf_tensor([ROWS, 1], f32))
        ndt = ctx.enter_context(nc.sbuf_tensor([ROWS, 3], f32))     # [0,0,nd]
        dden = ctx.enter_context(nc.sbuf_tensor([ROWS, 2], f32))    # [0,1]
        scd = ctx.enter_context(nc.sbuf_tensor([ROWS, 2], f32))     # den scan out
        scn = ctx.enter_context(nc.sbuf_tensor([ROWS, 3], f32))     # num scan out
        r = ctx.enter_context(nc.sbuf_tensor([ROWS, 1], f32))
        v1 = ctx.enter_context(nc.sbuf_tensor([ROWS, 1], f32))
        s_dma = ctx.enter_context(nc.semaphore("s_dma"))
        s_dve = ctx.enter_context(nc.semaphore("s_dve"))
        s_act = ctx.enter_context(nc.semaphore("s_act"))
        s_gp = ctx.enter_context(nc.semaphore("s_gp"))
        s_r = ctx.enter_context(nc.semaphore("s_r"))
        s_v = ctx.enter_context(nc.semaphore("s_v"))
        block = ctx.enter_context(nc.Block())

        @block.sync
        def _(sync):
            # final store: wait for the vector chain's last inc
            sync.wait_ge(s_dve, 2)
            sync.dma_start(out=xo_d[:, :], in_=xo[:, :]).then_inc(s_dma, 16)
            sync.wait_ge(s_dma, 48)

        @block.gpsimd
        def _(gpsimd):
            gpsimd.dma_start(out=hdr[:, :], in_=hd_d[:, :]).then_inc(s_dma, 16)
            gpsimd.dma_start(out=wp[:, :, :], in_=wp_d[:, :, :]).then_inc(s_dma, 16)

        # NOTE: DVE/ACT pipelines do not interlock same-engine RAW hazards in
        # raw bass -- a dependent back-to-back op reads stale SBUF.  Every
        # producer->consumer edge needs a drain() (pipeline flush) between.
        @block.vector
        def _(vector):
            nc.vector.memset(v[:, :], 0.0)
            nc.vector.memset(c[:, :], 0.0)
            nc.vector.memset(ndt[:, :], 0.0)
            nc.vector.memset(dden[:, 0:1], 0.0)
            nc.vector.memset(dden[:, 1:2], 1.0)
            nc.vector.drain().then_inc(s_dve, 1)  # c_0 = 0 / const tiles ready
            vector.wait_ge(s_dma, 16)  # header (ytt/btt/sat/ugt) landed
            for i in range(D):
                if 1 <= i <= D - 2:
                    # speculative partial-dot multiply for row i+1; runs under
                    # tanh_i (column i of v is still zero).  The free-axis sum
                    # happens on the otherwise-idle ScalarE.
                    if i == 1:
                        vector.wait_ge(s_dma, 32)  # W' landed
                    if i >= 2:
                        vector.wait_ge(s_r, i - 1)  # ScalarE consumed prod row i
                    nc.vector.tensor_mul(prod[:, :], v[:, :], wp[:, i + 1, :])
                    nc.vector.drain().then_inc(s_gp, 1)
                vector.wait_ge(s_act, i + 1)  # tanh_i + nd affine done
                # count = #{u_k < nd} + SEED  (exact fp32 integer count)
                nc.vector.tensor_scalar(
                    out=gs[:, :], in0=ugt[:, :], scalar1=ndt[:, 2:3],
                    scalar2=SEED, op0=Alu.is_lt, op1=Alu.add,
                    accum_out=cnt[:, :])
                nc.vector.drain()
                # Newton round 1 in count units (v0 = cnt*H1); Horner scans:
                #   den = (3*H1^2*cnt)*cnt + 1 ; num = ((2*H1^3*cnt)*cnt)*cnt + nd
                nc.vector.tensor_tensor_scan(
                    out=scd[:, :], data0=frep(cnt[:, 0:1], 2), data1=dden[:, :],
                    initial=float(3 * H1 * H1), op0=Alu.mult, op1=Alu.add)
                nc.vector.drain()
                nc.vector.reciprocal(out=r[:, :], in_=scd[:, 1:2])
                nc.vector.tensor_tensor_scan(
                    out=scn[:, :], data0=frep(cnt[:, 0:1], 3), data1=ndt[:, :],
                    initial=float(2 * H1 ** 3), op0=Alu.mult, op1=Alu.add)
                nc.vector.drain()
                nc.vector.tensor_mul(v1[:, :], scn[:, 2:3], r[:, :])
                nc.vector.drain()
                # Newton round 2 -> write v[:, i]
                nc.vector.tensor_tensor_scan(
                    out=scd[:, :], data0=frep(v1[:, 0:1], 2), data1=dden[:, :],
                    initial=3.0, op0=Alu.mult, op1=Alu.add)
                nc.vector.drain()
                nc.vector.reciprocal(out=r[:, :], in_=scd[:, 1:2])
                nc.vector.tensor_tensor_scan(
                    out=scn[:, :], data0=frep(v1[:, 0:1], 3), data1=ndt[:, :],
                    initial=2.0, op0=Alu.mult, op1=Alu.add)
                nc.vector.drain()
                nc.vector.tensor_mul(v[:, i:i + 1], scn[:, 2:3], r[:, :])
                if i <= D - 2:
                    nc.vector.drain().then_inc(s_v, 1)
                else:
                    nc.vector.drain()
            nc.vector.tensor_mul(xo[:, :], v[:, :], sat[:, :])
            nc.vector.drain().then_inc(s_dve, 1)

        @block.scalar
        def _(scalar):
            scalar.wait_ge(s_dma, 16)  # header landed
            for i in range(D):
                if i >= 2:
                    # cb = (partial dot of row i) + b_i : Copy+accum with the
                    # per-element bias b_i/D so the sum carries the tanh bias.
                    scalar.wait_ge(s_gp, i - 1)
                    nc.scalar.activation(
                        out=junk[:, :], in_=prod[:, :], func=Act.Copy,
                        bias=float(b64[i] / D), scale=1.0,
                        accum_out=cb[:, :])
                    nc.scalar.drain().then_inc(s_r, 1)
                # tanh_i; the last dot term W'[i,i-1]*v_{i-1} rides the scale
                if i == 0:
                    scalar.wait_ge(s_dve, 1)
                    nc.scalar.activation(
                        out=t[:, :], in_=c[:, :], func=Act.Tanh,
                        bias=btt[:, 0:1], scale=1.0)
                elif i == 1:
                    scalar.wait_ge(s_v, 1)
                    nc.scalar.activation(
                        out=t[:, :], in_=v[:, 0:1], func=Act.Tanh,
                        bias=btt[:, 1:2], scale=float(Wp[1, 0]))
                else:
                    scalar.wait_ge(s_v, i)
                    nc.scalar.activation(
                        out=t[:, :], in_=v[:, i - 1:i], func=Act.Tanh,
                        bias=cb[:, :], scale=float(Wp[i, i - 1]))
                nc.scalar.drain()
                # nd = Yt[:,i] - kappa_i * tanh(...), written into ndt[:,2]
                nc.scalar.activation(
                    out=ndt[:, 2:3], in_=t[:, :], func=Act.Identity,
                    bias=ytt[:, i:i + 1], scale=float(-kappa[i]))
                nc.scalar.drain().then_inc(s_act, 1)

    in_maps = []
    for c0 in range(NCORES):
        hdr_np = np.concatenate([
            Yt[c0 * ROWS:(c0 + 1) * ROWS],
            np.broadcast_to(BT, (ROWS, D)),
            np.broadcast_to(SA, (ROWS, D)),
            np.broadcast_to(UG, (ROWS, N1)),
        ], axis=1)
        in_maps.append({"hdr": np.ascontiguousarray(hdr_np), "wpb": WPB})
    return nc, in_maps


def kernel(y, W, s, b):
    from concourse.bass_utils import run_bass_kernel_spmd

    nc, in_maps = build(y, W, s, b)
    res = run_bass_kernel_spmd(nc, in_maps, list(range(NCORES))).results
    X = np.concatenate([res[c]["xout"] for c in range(NCORES)], axis=0)
    return X.astype(np.float32)


if __name__ == "__main__":
    rng = np.random.default_rng(0)
    y = rng.standard_normal((B, D)).astype(np.float32)
    W = np.tril(rng.standard_normal((D, D)), -1).astype(np.float32) * 0.5
    s = rng.standard_normal(D).astype(np.float32)
    b = rng.standard_normal(D).astype(np.float32)
    X = kernel(y=y, W=W, s=s, b=b)
    print("out", X.shape, X.dtype, X[0, :4])


# revision 37
# speedup vs baseline: 1.0538x; 1.0006x over previous
"""Trainium2 Bass kernel for nn_AutoregressiveBisectionInverter.

Inverts y = softplus(s)*x + 0.1*x^3 + tanh(W@x + b) (W strictly lower
triangular) per batch row.  Since W is strictly lower-triangular, the tanh
term at position i depends only on already-solved x_{<i}; each position is
a monotone-cubic scalar root solve.

Strategy (per NeuronCore, batch sharded 1024 -> 8 x 128 rows on the 128
SBUF partitions):
  - Normalize:  x = sqrt(abar)*v with abar = 10*softplus(s)  so the cubic
    becomes p(v) = v^3 + v + dt  (unit coefficients, p' >= 1, |root| <= VM).
  - Per autoregressive step i (serial DVE chain + ScalarE leg):
      ScalarE: tanh_i = Tanh(W'[i,i-1]*v_{i-1} + cb)  -- the last dot term
               rides tanh's scale, cb = (partial dot + b_i) comes from a
               Copy+accum_out reduce seeded with bias=b_i/D;
               nd = Yt[:,i] - kappa_i*tanh_i  (Identity activation)
      DVE:  cnt = #{k: u_k < nd} + seed   (ONE tensor_scalar is_lt+accum over
              a host-baked grid u_k = p-poly(v_k); exact fp32 count ~ 7
              bisection steps)
            two Newton polish rounds, each as: Horner scan (den), reciprocal,
            Horner scan (num), multiply -- tensor_tensor_scan with a
            stride-0 free-axis broadcast of v evaluates 3v^2+1 and 2v^3+nd
            in one instruction each; round 1 runs in count units with the
            grid pitch h1 folded into the scan initial values.
      The [128,32] partial-dot multiply for row i+1 runs on DVE during step
      i's tanh window (column i of v is still zero there).
  - Output x = sqrt(abar)*v (one elementwise mult), DMA out.

Raw bass Blocks are used (TileContext's tail drain trips a sync-wait limit
in this walrus build), with explicit drain() between every same-engine
producer->consumer pair (DVE/ACT pipelines do not interlock RAW hazards).
All input-dependent scalars are baked as instruction immediates at trace
time; broadcasts/grids are precomputed on the host and DMA'd in dependency
order so compute starts after the first small loads.
"""

import numpy as np

B, D = 1024, 32
NCORES = 8
ROWS = B // NCORES  # 128 rows per core == SBUF partitions
N1 = 128            # bisection-grid points in the fused count op


def _softplus64(x):
    x = x.astype(np.float64)
    return np.log1p(np.exp(-np.abs(x))) + np.maximum(x, 0)


def build(y, W, s, b):
    """Build the SPMD Bass program; returns (nc, in_maps)."""
    from contextlib import ExitStack
    import concourse.bass as bass
    from concourse import mybir

    f32 = mybir.dt.float32
    Alu = mybir.AluOpType
    Act = mybir.ActivationFunctionType

    y = np.ascontiguousarray(np.asarray(y), dtype=np.float32)
    W64 = np.asarray(W, dtype=np.float64)
    s64 = np.asarray(s, dtype=np.float64)
    b64 = np.asarray(b, dtype=np.float64)

    # ---- host precompute ----
    abar = 10.0 * _softplus64(s64)                 # v-linear coefficient
    sqrt_abar = np.sqrt(abar)
    kappa = (10.0 * abar ** -1.5).astype(np.float32)     # per-step immediates
    Yt = (10.0 * y.astype(np.float64) * abar[None, :] ** -1.5).astype(np.float32)
    Wp = np.ascontiguousarray((W64 * sqrt_abar[None, :]).astype(np.float32))
    SA = sqrt_abar.astype(np.float32)[None, :]            # [1, D]
    BT = b64.astype(np.float32)[None, :]                  # [1, D] tanh bias

    dmax = 10.0 * (1.0 + np.abs(y).max(axis=0)) * abar ** -1.5
    VM = float(np.max(np.minimum(np.cbrt(dmax), dmax)) * 1.02 + 1e-3)
    H1 = float(np.float32(2 * VM / (N1 - 1)))
    VM = float(np.float32(VM))
    vk = (-VM + np.arange(N1, dtype=np.float64) * H1)
    UG = ((vk * vk + 1.0) * vk).astype(np.float32)[None, :]   # [1, N1] p-poly
    SEED = float(np.float32(-VM / H1 - 0.5))  # v0 = (count + SEED) * H1

    # One header array per core: [ ytt | btt | sat | ugt ] columns, plus a
    # pre-broadcast W' -- exactly two input DMAs (DMA cost here is dominated
    # by the 128 per-partition descriptors, not bytes).
    HW = 3 * D + N1
    WPB = np.ascontiguousarray(np.broadcast_to(Wp[None, :, :], (ROWS, D, D)))

    # ---- build the SPMD Bass program (input-dependent immediates baked) ----
    nc = bass.Bass()
    hd_d = nc.dram_tensor("hdr", [ROWS, HW], f32, kind="ExternalInput")
    wp_d = nc.dram_tensor("wpb", [ROWS, D, D], f32, kind="ExternalInput")
    xo_d = nc.dram_tensor("xout", [ROWS, D], f32, kind="ExternalOutput")

    def frep(ap, k):
        # broadcast a [P,1] AP along the free axis via stride 0
        return bass.AP(tensor=ap.tensor, offset=ap.offset,
                       ap=[list(ap.ap[0]), [0, k]])

    with ExitStack() as ctx:
        v = ctx.enter_context(nc.sbuf_tensor([ROWS, D], f32))       # v-space solution
        wp = ctx.enter_context(nc.sbuf_tensor([ROWS, D, D], f32))   # W' bcast
        hdr = ctx.enter_context(nc.sbuf_tensor([ROWS, HW], f32))
        ytt = hdr[:, 0:D]
        btt = hdr[:, D:2 * D]
        sat = hdr[:, 2 * D:3 * D]
        ugt = hdr[:, 3 * D:3 * D + N1]
        xo = ctx.enter_context(nc.sbuf_tensor([ROWS, D], f32))
        gs = ctx.enter_context(nc.sbuf_tensor([ROWS, N1], f32))     # count scratch
        prod = ctx.enter_context(nc.sbuf_tensor([ROWS, D], f32))
        junk = ctx.enter_context(nc.sbuf_tensor([ROWS, D], f32))
        c = ctx.enter_context(nc.sbuf_tensor([ROWS, 1], f32))
        t = ctx.enter_context(nc.sbuf_tensor([ROWS, 1], f32))
        cb = ctx.enter_context(nc.sbuf_tensor([ROWS, 1], f32))      # cpart + b_i
        cnt = ctx.enter_context(nc.sbuf_tensor([ROWS, 1], f32))
        ndt = ctx.enter_context(nc.sbuf_tensor([ROWS, 3], f32))     # [0,0,nd]
        dden = ctx.enter_context(nc.sbuf_tensor([ROWS, 2], f32))    # [0,1]
        scd = ctx.enter_context(nc.sbuf_tensor([ROWS, 2], f32))     # den scan out
        scn = ctx.enter_context(nc.sbuf_tensor([ROWS, 3], f32))     # num scan out
        r = ctx.enter_context(nc.sbuf_tensor([ROWS, 1], f32))
        v1 = ctx.enter_context(nc.sbuf_tensor([ROWS, 1], f32))
        s_dma = ctx.enter_context(nc.semaphore("s_dma"))
        s_dve = ctx.enter_context(nc.semaphore("s_dve"))
        s_act = ctx.enter_context(nc.semaphore("s_act"))
        s_gp = ctx.enter_context(nc.semaphore("s_gp"))
        s_r = ctx.enter_context(nc.semaphore("s_r"))
        s_v = ctx.enter_context(nc.semaphore("s_v"))
        block = ctx.enter_context(nc.Block())

        @block.sync
        def _(sync):
            # final store: wait for the vector chain's last inc
            sync.wait_ge(s_dve, 2)
            sync.dma_start(out=xo_d[:, :], in_=xo[:, :]).then_inc(s_dma, 16)
            sync.wait_ge(s_dma, 48)

        @block.gpsimd
        def _(gpsimd):
            gpsimd.dma_start(out=hdr[:, :], in_=hd_d[:, :]).then_inc(s_dma, 16)
            gpsimd.dma_start(out=wp[:, :, :], in_=wp_d[:, :, :]).then_inc(s_dma, 16)

        # NOTE: DVE/ACT pipelines do not interlock same-engine RAW hazards in
        # raw bass -- a dependent back-to-back op reads stale SBUF.  Every
        # producer->consumer edge needs a drain() (pipeline flush) between.
        @block.vector
        def _(vector):
            nc.vector.memset(v[:, :], 0.0)
            nc.vector.memset(c[:, :], 0.0)
            nc.vector.memset(ndt[:, :], 0.0)
            nc.vector.memset(dden[:, 0:1], 0.0)
            nc.vector.memset(dden[:, 1:2], 1.0)
            nc.vector.drain().then_inc(s_dve, 1)  # c_0 = 0 / const tiles ready
            vector.wait_ge(s_dma, 16)  # header (ytt/btt/sat/ugt) landed
            for i in range(D):
                if 1 <= i <= D - 2:
                    # speculative partial-dot multiply for row i+1; runs under
                    # tanh_i (column i of v is still zero).  The free-axis sum
                    # happens on the otherwise-idle ScalarE.
                    if i == 1:
                        vector.wait_ge(s_dma, 32)  # W' landed
                    if i >= 2:
                        vector.wait_ge(s_r, i - 1)  # ScalarE consumed prod row i
                    nc.vector.tensor_mul(prod[:, :], v[:, :], wp[:, i + 1, :])
                    nc.vector.drain().then_inc(s_gp, 1)
                vector.wait_ge(s_act, i + 1)  # tanh_i + nd affine done
                # count = #{u_k < nd} + SEED  (exact fp32 integer count)
                nc.vector.tensor_scalar(
                    out=gs[:, :], in0=ugt[:, :], scalar1=ndt[:, 2:3],
                    scalar2=SEED, op0=Alu.is_lt, op1=Alu.add,
                    accum_out=cnt[:, :])
                nc.vector.drain()
                # Newton round 1 in count units (v0 = cnt*H1); Horner scans:
                #   den = (3*H1^2*cnt)*cnt + 1 ; num = ((2*H1^3*cnt)*cnt)*cnt + nd
                nc.vector.tensor_tensor_scan(
                    out=scd[:, :], data0=frep(cnt[:, 0:1], 2), data1=dden[:, :],
                    initial=float(3 * H1 * H1), op0=Alu.mult, op1=Alu.add)
                nc.vector.drain()
                nc.vector.reciprocal(out=r[:, :], in_=scd[:, 1:2])
                nc.vector.tensor_tensor_scan(
                    out=scn[:, :], data0=frep(cnt[:, 0:1], 3), data1=ndt[:, :],
                    initial=float(2 * H1 ** 3), op0=Alu.mult, op1=Alu.add)
                nc.vector.drain()
                nc.vector.tensor_mul(v1[:, :], scn[:, 2:3], r[:, :])
                nc.vector.drain()
                # Newton round 2 -> write v[:, i]
                nc.vector.tensor_tensor_scan(
                    out=scd[:, :], data0=frep(v1[:, 0:1], 2), data1=dden[:, :],
                    initial=3.0, op0=Alu.mult, op1=Alu.add)
                nc.vector.drain()
                nc.vector.reciprocal(out=r[:, :], in_=scd[:, 1:2])
                nc.vector.tensor_tensor_scan(
                    out=scn[:, :], data0=frep(v1[:, 0:1], 3), data1=ndt[:, :],
                    initial=2.0, op0=Alu.mult, op1=Alu.add)
                nc.vector.drain()
                nc.vector.tensor_mul(v[:, i:i + 1], scn[:, 2:3], r[:, :])
                if i <= D - 2:
                    nc.vector.drain().then_inc(s_v, 1)
                else:
                    nc.vector.drain()
            nc.vector.tensor_mul(xo[:, :], v[:, :], sat[:, :])
            nc.vector.drain().then_inc(s_dve, 1)

        @block.scalar
        def _(scalar):
            scalar.wait_ge(s_dma, 16)  # header landed
            for i in range(D):
                if i >= 2:
                    # cb = (partial dot of row i) + b_i : Copy+accum with the
                    # per-element bias b_i/D so the sum carries the tanh bias.
                    scalar.wait_ge(s_gp, i - 1)
                    nc.scalar.activation(
                        out=junk[:, :], in_=prod[:, :], func=Act.Copy,
                        bias=float(b64[i] / D), scale=1.0,
                        accum_out=cb[:, :])
                    nc.scalar.drain().then_inc(s_r, 1)
                # tanh_i; the last dot term W'[i,i-1]*v_{i-1} rides the scale
                if i == 0:
                    scalar.wait_ge(s_dve, 1)
                    nc.scalar.activation(
                        out=t[:, :], in_=c[:, :], func=Act.Tanh,
                        bias=btt[:, 0:1], scale=1.0)
                elif i == 1:
                    scalar.wait_ge(s_v, 1)
                    nc.scalar.activation(
                        out=t[:, :], in_=v[:, 0:1], func=Act.Tanh,
                        bias=btt[:, 1:2], scale=float(Wp[1, 0]))
                else:
                    scalar.wait_ge(s_v, i)
                    nc.scalar.activation(
                        out=t[:, :], in_=v[:, i - 1:i], func=Act.Tanh,
                        bias=cb[:, :], scale=float(Wp[i, i - 1]))
                # ScalarE exposes its read-write bubble between instructions
                # (unlike DVE) -- adjacent ACT->ACT RAW needs no drain.
                # nd = Yt[:,i] - kappa_i * tanh(...), written into ndt[:,2]
                nc.scalar.activation(
                    out=ndt[:, 2:3], in_=t[:, :], func=Act.Identity,
                    bias=ytt[:, i:i + 1], scale=float(-kappa[i]))
                nc.scalar.drain().then_inc(s_act, 1)

    in_maps = []
    for c0 in range(NCORES):
        hdr_np = np.concatenate([
            Yt[c0 * ROWS:(c0 + 1) * ROWS],
            np.broadcast_to(BT, (ROWS, D)),
            np.broadcast_to(SA, (ROWS, D)),
            np.broadcast_to(UG, (ROWS, N1)),
        ], axis=1)
        in_maps.append({"hdr": np.ascontiguousarray(hdr_np), "wpb": WPB})
    return nc, in_maps


def kernel(y, W, s, b):
    from concourse.bass_utils import run_bass_kernel_spmd

    nc, in_maps = build(y, W, s, b)
    res = run_bass_kernel_spmd(nc, in_maps, list(range(NCORES))).results
    X = np.concatenate([res[c]["xout"] for c in range(NCORES)], axis=0)
    return X.astype(np.float32)


if __name__ == "__main__":
    rng = np.random.default_rng(0)
    y = rng.standard_normal((B, D)).astype(np.float32)
    W = np.tril(rng.standard_normal((D, D)), -1).astype(np.float32) * 0.5
    s = rng.standard_normal(D).astype(np.float32)
    b = rng.standard_normal(D).astype(np.float32)
    X = kernel(y=y, W=W, s=s, b=b)
    print("out", X.shape, X.dtype, X[0, :4])
